# revision 1
# baseline (speedup 1.0000x reference)
"""Trainium2 Bass kernel for windowed sigmoid-attention (nn_Attention_24927990186215).

Reference computation (per full input):
    x: [16, 16, 16, 16, 512]  (b, nh, nw, t, d) -- windows of T=16 tokens
    q/k/v = x @ W{q,k,v} + b{q,k,v}; split into H=8 heads of 64
    scores = q @ k^T / sqrt(64) within each 16-token window
    probs = sigmoid(scores)            (elementwise, NOT softmax)
    ctx = probs @ v;  out = ctx @ Wo + bo

Sharding: data-parallel over batch dim (16) across 8 cores -> 2 batches
(8192 tokens) per core.

Per-core dataflow (all matmuls on the PE):
  - x is DMA'd in 512-token supergroups, transposed on the PE (via
    identity matmul) to get features on partitions (x^T).
  - q^T, k^T are computed feature-major (stationary = W chunk, moving =
    x^T) in fp32r (full PE rate at N=512); v is computed token-major
    (stationary = x^T chunk, moving = Wv).  1/sqrt(64) is folded into Wq
    on the host.
  - scores for a group of 8 windows (128 tokens) are computed as a dense
    [128,128] block per head (8x FLOP waste, but windows batch onto the
    PE); 4 heads share one [128,512] PSUM bank.  Sigmoid runs on the
    scalar engine (PSUM -> SBUF, cast to bf16), then a block-diagonal
    0/1 mask multiply on the vector engine zeroes the cross-window
    garbage.
  - ctx^T = (masked probs)^T-contraction against v, accumulated per
    head-pair into one [128,512] PSUM bank (col-packed heads), then the
    output projection runs token-major in fp32r and results are DMA'd
    out in natural layout.
  - attention-inner matmuls run in bf16 (scores accumulate in fp32 PSUM).

Biases are folded in only when nonzero (the spec fills them with zeros):
bq/bk ride the q^T/k^T PSUM->SBUF copy as per-partition activation
biases; bv/bo are added via rank-1 ones-row matmuls into the PSUM
accumulation.
"""

import numpy as np
import ml_dtypes

# ---- problem constants (hardcoded per the task contract) ----
N_CORES = 8
B, NH, NW, T, D = 16, 16, 16, 16, 512
HEADS, HS = 8, 64
TOK = (B // N_CORES) * NH * NW * T  # 8192 tokens per core
NG = TOK // 512                     # 16 supergroups of 512 tokens
SCALE = 1.0 / 8.0                   # 1/sqrt(HS)

_CACHE = {}
DEBUG_SKIP = set()  # dev-only: subset of {"scores", "ctx"}


def _build(n_cores, with_bq, with_bk, with_bv, with_bo):
    import concourse.bacc as bacc
    import concourse.mybir as mybir
    import concourse.tile as tile

    f32 = mybir.dt.float32
    f32r = mybir.dt.float32r
    bf16 = mybir.dt.bfloat16
    AFT = mybir.ActivationFunctionType

    nc = bacc.Bacc("TRN2", target_bir_lowering=False, debug=False,
                   num_devices=n_cores)

    x_d = nc.dram_tensor("xb", [TOK, D], bf16, kind="ExternalInput").ap()
    wqk_d = nc.dram_tensor("wqk", [2 * D, D], bf16,
                           kind="ExternalInput").ap()
    wv_d = nc.dram_tensor("wv", [D, D], bf16, kind="ExternalInput").ap()
    wo_d = nc.dram_tensor("wo", [D, D], f32r, kind="ExternalInput").ap()
    mask_d = nc.dram_tensor("mask4", [128, 512], bf16, kind="ExternalInput").ap()
    bias_d = {}
    for name, used, dt_b in (("bq", with_bq, f32), ("bk", with_bk, f32),
                             ("bv", with_bv, bf16), ("bo", with_bo, f32r)):
        if used:
            bias_d[name] = nc.dram_tensor(name, [D], dt_b,
                                          kind="ExternalInput").ap()
    y_d = nc.dram_tensor("y", [TOK, D], f32, kind="ExternalOutput").ap()

    with tile.TileContext(nc) as tc:
        with (
            tc.tile_pool(name="const", bufs=1) as cpool,
            tc.tile_pool(name="xin", bufs=2) as xpool,
            tc.tile_pool(name="work", bufs=2) as wpool,
            tc.tile_pool(name="psum", bufs=4, space="PSUM") as ppool,
        ):
            # ---- constants ----
            wsb = {}
            wqk_t = cpool.tile([128, 8 * 512], bf16, name="wqk_sb")
            nc.scalar.dma_start(
                out=wqk_t.rearrange("p (c f) -> p c f", c=8),
                in_=wqk_d.rearrange("(c p) f -> p c f", p=128))
            wsb["wq"] = wqk_t[:, :4 * 512]
            wsb["wk"] = wqk_t[:, 4 * 512:]
            for name, d_ap, dt_w in (("wv", wv_d, bf16), ("wo", wo_d, f32r)):
                w_t = cpool.tile([128, 4 * 512], dt_w, name=f"{name}_sb")
                nc.scalar.dma_start(
                    out=w_t.rearrange("p (c f) -> p c f", c=4),
                    in_=d_ap.rearrange("(c p) f -> p c f", p=128))
                wsb[name] = w_t
            mask_sb = cpool.tile([128, 512], bf16, name="mask_sb")
            nc.scalar.dma_start(out=mask_sb[:], in_=mask_d[:])
            bias_sb = {}
            for name, ap_d in bias_d.items():
                if name not in ("bq", "bk"):
                    continue
                b_t = cpool.tile([128, 4], f32, name=f"{name}_sb")
                # chunk c of the bias vector in column c (partition = feature)
                nc.scalar.dma_start(
                    out=b_t[:],
                    in_=ap_d.rearrange("(c p) -> p c", p=128))
                bias_sb[name] = b_t
            ones_sb = ones_bf_sb = None
            if with_bo:
                ones_sb = cpool.tile([1, 128], f32r, name="ones_sb")
                nc.gpsimd.memset(ones_sb[:], 1.0)
            if with_bv:
                ones_bf_sb = cpool.tile([1, 128], bf16, name="ones_bf_sb")
                nc.gpsimd.memset(ones_bf_sb[:], 1.0)
            # row-vector copies of bv / bo for rank-1 bias matmuls
            bvrow_sb = bohrow_sb = None
            if with_bv:
                bvrow_sb = cpool.tile([1, 512], bf16, name="bvrow_sb")
                nc.scalar.dma_start(out=bvrow_sb[:],
                                    in_=bias_d["bv"].unsqueeze(0))
            if with_bo:
                bohrow_sb = cpool.tile([1, 512], f32r, name="bohrow_sb")
                nc.scalar.dma_start(out=bohrow_sb[:],
                                    in_=bias_d["bo"].unsqueeze(0))

            def r(ap):
                return ap.bitcast(f32r)

            # ---- per-supergroup emitters (2-stage software pipeline) ----
            def load_xt(G):
                """DMA-transpose x rows (bf16) straight into feature-major
                SBUF chunks."""
                xt = [wpool.tile([128, 512], bf16, name=f"xt{c}",
                                 tag=f"xt{c}") for c in range(4)]
                for c in range(4):
                    nc.sync.dma_start_transpose(
                        xt[c][:],
                        x_d[G * 512:(G + 1) * 512, c * 128:(c + 1) * 128])
                return xt

            def proj_qk_chunk(G, xt, wname, bname, dst, c):
                w_t = wsb[wname]
                pj_ps = ppool.tile([128, 512], f32, name="pj_ps", tag="ps")
                for k in range(4):
                    nc.tensor.matmul(
                        pj_ps[:],
                        w_t[:, k * 512 + c * 128:
                            k * 512 + (c + 1) * 128],
                        xt[k][:],
                        start=(k == 0), stop=(k == 3))
                if bname in bias_sb:
                    nc.scalar.activation(
                        dst[c][:], pj_ps[:], AFT.Identity,
                        bias=bias_sb[bname][:, c:c + 1])
                elif (wname == "wq") == (c % 2 == 0):
                    nc.vector.tensor_copy(dst[c][:], pj_ps[:])
                else:
                    nc.scalar.copy(dst[c][:], pj_ps[:])

            def proj_v(G, xt):
                v = [wpool.tile([128, 512], bf16, name=f"v{g}", tag=f"v{g}")
                     for g in range(4)]
                for g in range(4):
                    v_ps = ppool.tile([128, 512], f32, name="v_ps", tag="ps")
                    for k in range(4):
                        nc.tensor.matmul(
                            v_ps[:],
                            xt[k][:, g * 128:(g + 1) * 128],
                            wsb["wv"][:, k * 512:(k + 1) * 512],
                            start=(k == 0), stop=(k == 3 and not with_bv))
                    if with_bv:
                        nc.tensor.matmul(v_ps[:], ones_bf_sb[:],
                                         bvrow_sb[:],
                                         start=False, stop=True)
                    if g % 2 == 0:
                        nc.vector.tensor_copy(v[g][:], v_ps[:])
                    else:
                        nc.scalar.copy(v[g][:], v_ps[:])
                return v

            def scores(P, qt, kt, g):
                """S' matmuls + sigmoid + mask for one 128-token group."""
                p4 = []
                for half in range(2):  # even heads / odd heads
                    # one bank takes a uniform stationary base partition:
                    # mixing base 0/64 row-groups within a bank crashes NRT
                    s_ps = ppool.tile([128, 512], f32, name="s_ps", tag="s",
                                      bufs=4)
                    lo = half * 64
                    for hh in range(4):
                        h = 2 * hh + half
                        c = h // 2
                        gcols = slice(g * 128, (g + 1) * 128)
                        nc.tensor.matmul(
                            s_ps[:, hh * 128:(hh + 1) * 128],
                            kt[c][lo:lo + 64, gcols],
                            qt[c][lo:lo + 64, gcols],
                            start=True, stop=True)
                    p_t = wpool.tile([128, 512], bf16, name=f"p{g}_{half}",
                                     tag=f"p{g}_{half}")
                    if "sig" in DEBUG_SKIP:
                        nc.vector.tensor_copy(p_t[:], s_ps[:])
                    else:
                        nc.scalar.activation(p_t[:], s_ps[:], AFT.Sigmoid)
                    if "mask" in DEBUG_SKIP:
                        return [p_t, p_t]
                    nc.vector.tensor_mul(
                        p_t.rearrange("p (hh t) -> p hh t", hh=4),
                        p_t.rearrange("p (hh t) -> p hh t", hh=4),
                        mask_sb.rearrange("p (hh t) -> p hh t", hh=4))
                    p4.append(p_t)
                return p4

            def ctx_out(P, pr, v):
                ctxt = []
                for g in range(4):
                    ctx_ps = ppool.tile([128, 512], f32, name="ctx_ps",
                                        tag="ps")
                    for h in range(HEADS):
                        c, lo = h // 2, (h % 2) * 64
                        nc.tensor.matmul(
                            ctx_ps[lo:lo + 64, c * 128:(c + 1) * 128],
                            v[g][:, h * 64:(h + 1) * 64],
                            pr[g][h % 2][:, (h // 2) * 128:
                                          (h // 2 + 1) * 128],
                            start=True, stop=True)
                    ctx_t = wpool.tile([128, 512], f32r, name="ctx_t",
                                       tag=f"ctx_t{g}", bufs=2)
                    if g % 2 == 0:
                        nc.scalar.copy(ctx_t[:], ctx_ps[:])
                    else:
                        nc.vector.tensor_copy(ctx_t[:], ctx_ps[:])
                    ctxt.append(ctx_t)
                for g in range(4):
                    o_ps = ppool.tile([128, 512], f32, name="o_ps", tag="ps")
                    for c in range(4):
                        nc.tensor.matmul(
                            o_ps[:],
                            ctxt[g][:, c * 128:(c + 1) * 128],
                            wsb["wo"][:, c * 512:(c + 1) * 512],
                            start=(c == 0), stop=(c == 3 and not with_bo))
                    if with_bo:
                        nc.tensor.matmul(o_ps[:], ones_sb[:],
                                         bohrow_sb[:],
                                         start=False, stop=True)
                    o_t = wpool.tile([128, 512], f32, name="o_t", tag="o_t",
                                     bufs=4)
                    if g % 2 == 0:
                        nc.scalar.copy(o_t[:], o_ps[:])
                    else:
                        nc.vector.tensor_copy(o_t[:], o_ps[:])
                    nc.scalar.dma_start(
                        out=y_d[(P * 4 + g) * 128:(P * 4 + g + 1) * 128, :],
                        in_=o_t[:])

            # ---- pipelined emission: stage A(G) interleaved with B(G-1) ----
            xt_next = load_xt(0)
            prev = None  # (P, qt, kt, v)
            for G in range(NG + 1):
                xt = xt_next
                if G + 1 < NG:
                    xt_next = load_xt(G + 1)
                pr = []
                if G < NG:
                    qt = [wpool.tile([128, 512], bf16, name=f"wqt{c}",
                                     tag=f"wqt{c}") for c in range(4)]
                    kt = [wpool.tile([128, 512], bf16, name=f"wkt{c}",
                                     tag=f"wkt{c}") for c in range(4)]
                for g in range(4):
                    if prev is not None and "scores" not in DEBUG_SKIP:
                        pr.append(scores(prev[0], prev[1], prev[2], g))
                    if G < NG:
                        proj_qk_chunk(G, xt, "wq", "bq", qt, g)
                        proj_qk_chunk(G, xt, "wk", "bk", kt, g)
                if G < NG:
                    v = proj_v(G, xt)
                if prev is not None and pr and "ctx" not in DEBUG_SKIP:
                    ctx_out(prev[0], pr, prev[3])
                prev = (G, qt, kt, v) if G < NG else None

    nc.compile()
    return nc


def _get_nc(n_cores, flags):
    key = (n_cores, flags)
    if key not in _CACHE:
        _CACHE[key] = _build(n_cores, *flags)
    return _CACHE[key]


def _mask4():
    m = np.zeros((128, 128), dtype=ml_dtypes.bfloat16)
    for w in range(8):
        m[w * 16:(w + 1) * 16, w * 16:(w + 1) * 16] = 1
    return np.ascontiguousarray(np.tile(m, (1, 4)))


def kernel(x, Wq, bq, Wk, bk, Wv, bv, Wo, bo):
    from concourse.bass_utils import run_bass_kernel_spmd

    in_dt = x.dtype
    flags = tuple(bool(np.any(b)) for b in (bq, bk, bv, bo))
    nc = _get_nc(N_CORES, flags)

    xf = np.ascontiguousarray(np.asarray(x, dtype=np.float32)
                              .reshape(N_CORES, TOK, D)
                              .astype(ml_dtypes.bfloat16))
    base = {
        "wqk": np.ascontiguousarray(np.concatenate(
            [np.asarray(Wq, np.float32) * SCALE,
             np.asarray(Wk, np.float32)], axis=0)
            .astype(ml_dtypes.bfloat16)),
        "wv": np.ascontiguousarray(np.asarray(Wv, np.float32)
                                   .astype(ml_dtypes.bfloat16)),
        "wo": np.ascontiguousarray(np.asarray(Wo, np.float32)),
        "mask4": _mask4(),
    }
    for name, b, used, scale in (("bq", bq, flags[0], SCALE),
                                 ("bk", bk, flags[1], 1.0),
                                 ("bv", bv, flags[2], 1.0),
                                 ("bo", bo, flags[3], 1.0)):
        if used:
            arr = np.asarray(b, np.float32) * scale
            if name == "bv":
                arr = arr.astype(ml_dtypes.bfloat16)
            base[name] = np.ascontiguousarray(arr)

    in_maps = [dict(base, xb=xf[i]) for i in range(N_CORES)]
    res = run_bass_kernel_spmd(nc, in_maps, list(range(N_CORES)))
    out = np.concatenate([res.results[i]["y"] for i in range(N_CORES)], axis=0)
    return out.reshape(B, NH, NW, T, D).astype(in_dt, copy=False)



# revision 13
# speedup vs baseline: 1.2316x; 1.2316x over previous
"""Trainium2 Bass kernel for windowed sigmoid-attention (nn_Attention_24927990186215).

Reference computation (per full input):
    x: [16, 16, 16, 16, 512]  (b, nh, nw, t, d) -- windows of T=16 tokens
    q/k/v = x @ W{q,k,v} + b{q,k,v}; split into H=8 heads of 64
    scores = q @ k^T / sqrt(64) within each 16-token window
    probs = sigmoid(scores)  (elementwise, NOT softmax)
    ctx = probs @ v;  out = ctx @ Wo + bo
Sharding: data-parallel over batch dim (16) across 8 cores -> 2 batches
(8192 tokens) per core.

Per-core dataflow (all matmuls on the PE):
  - the host pre-transposes x and ships three feature-major copies: bf16
    (for the v projection) and an fp8e4m3 hi/lo residual pair (x ~ xh + xl)
    for the q/k projections.  No on-device transposes are needed.
  - q^T/k^T are computed feature-major with fp8 DoubleRow matmuls (two
    128-row contraction tiles per pass, 2x PE throughput): three residual
    cross terms xh*Wh + xh*Wl + xl*Wh (the xl*Wl term is ~0.1% and is
    dropped), 6 DoubleRow matmuls per 512-wide chunk vs 4 bf16 matmuls --
    25% fewer PE cycles at ~0.2% error.  Weights are pre-scaled (x32) on
    the host to keep fp8 out of the subnormal range; the q copy un-scales
    by 1/1024.
  - v is computed token-major in bf16 (stationary = x^T chunk); Wv carries
    a 1/16 scale so ctx lands in good fp8 range for the output projection.
  - scores for a group of 8 windows (128 tokens) are computed as a dense
    [128,128] block per head; 4 heads share one [128,512] PSUM bank.
    Sigmoid runs on the scalar engine (cast to bf16), then a block-diagonal
    0/1 mask multiply on the vector engine zeroes cross-window garbage.
  - ctx^T = (masked probs)^T-contraction against v, accumulated per
    head-pair into one [128,512] PSUM bank (col-packed heads).  The
    PSUM->SBUF step emits an fp8 hi copy plus an fp8 residual lo
    (vector-engine subtract), laid out in DoubleRow stationary form.
  - the output projection runs as 6 fp8 DoubleRow matmuls per group
    (ctx_h*Woh + ctx_h*Wol + ctx_l*Woh, Wo pre-scaled x16 to cancel v's
    1/16); y is written to DRAM in bf16 and cast to f32 on the host.

Biases are folded in only when nonzero (the spec fills them with zeros).
"""

import numpy as np
import ml_dtypes

# ---- problem constants (hardcoded per the task contract) ----
N_CORES = 8
B, NH, NW, T, D = 16, 16, 16, 16, 512
HEADS, HS = 8, 64
TOK = (B // N_CORES) * NH * NW * T  # 8192 tokens per core
NG = TOK // 512                     # 16 supergroups of 512 tokens
SCALE = 1.0 / 8.0                   # 1/sqrt(HS)
WQK_S = 32.0                        # fp8 range scale on Wq/Wk
WV_S = 1.0 / 16.0                   # scale on Wv (ctx -> fp8 range)
WO_S = 16.0                         # scale on Wo (cancels WV_S)
PM = 144                            # padded DoubleRow stationary stride
PN = 528                            # padded DoubleRow moving stride

_CACHE = {}


def _build(n_cores, with_bq, with_bk, with_bv, with_bo):
    import concourse.bacc as bacc
    import concourse.mybir as mybir
    import concourse.tile as tile

    f32 = mybir.dt.float32
    f32r = mybir.dt.float32r
    bf16 = mybir.dt.bfloat16
    fp8 = mybir.dt.float8e4
    AFT = mybir.ActivationFunctionType
    DR = mybir.MatmulPerfMode.DoubleRow
    SUB = mybir.AluOpType.subtract

    nc = bacc.Bacc("TRN2", target_bir_lowering=False, debug=False,
                   num_devices=n_cores)

    xh_d = nc.dram_tensor("xh", [D, TOK], fp8, kind="ExternalInput").ap()
    xl_d = nc.dram_tensor("xl", [D, TOK], fp8, kind="ExternalInput").ap()
    w8_d = {}
    for name in ("wqh", "wql", "wkh", "wkl"):
        w8_d[name] = nc.dram_tensor(name, [128, 4 * 2 * 2 * PM], fp8,
                                    kind="ExternalInput").ap()
    for name in ("woh", "wol", "wvh", "wvl"):
        w8_d[name] = nc.dram_tensor(name, [128, 2 * 2 * PN], fp8,
                                    kind="ExternalInput").ap()
    mask_d = nc.dram_tensor("maskp", [128, 512], bf16,
                            kind="ExternalInput").ap()
    bias_d = {}
    for name, used, dt_b in (("bq", with_bq, f32), ("bk", with_bk, f32),
                             ("bv", with_bv, bf16), ("bo", with_bo, f32r)):
        if used:
            bias_d[name] = nc.dram_tensor(name, [D], dt_b,
                                          kind="ExternalInput").ap()
    y_d = nc.dram_tensor("y", [TOK, D], bf16, kind="ExternalOutput").ap()

    with tile.TileContext(nc) as tc:
        with (
            tc.tile_pool(name="const", bufs=1) as cpool,
            tc.tile_pool(name="work", bufs=2) as wpool,
            tc.tile_pool(name="psum", bufs=4, space="PSUM") as ppool,
        ):
            # ---- constants ----
            w8 = {}
            for name in ("wqh", "wql", "wkh", "wkl", "woh", "wol",
                         "wvh", "wvl"):
                cols = 4 * 2 * 2 * PM if name[1] == "q" or name[1] == "k" \
                    else 2 * 2 * PN
                t_w = cpool.tile([128, cols], fp8, name=f"{name}_sb")
                nc.scalar.dma_start(out=t_w[:], in_=w8_d[name][:])
                w8[name] = t_w
            mask_sb = cpool.tile([128, 512], bf16, name="mask_sb")
            nc.scalar.dma_start(out=mask_sb[:], in_=mask_d[:])
            bias_sb = {}
            for name, ap_d in bias_d.items():
                if name not in ("bq", "bk"):
                    continue
                b_t = cpool.tile([128, 4], f32, name=f"{name}_sb")
                nc.scalar.dma_start(
                    out=b_t[:],
                    in_=ap_d.rearrange("(c p) -> p c", p=128))
                bias_sb[name] = b_t
            ones_sb = ones_bf_sb = None
            if with_bo:
                ones_sb = cpool.tile([1, 128], f32r, name="ones_sb")
                nc.gpsimd.memset(ones_sb[:], 1.0)
            if with_bv:
                ones_bf_sb = cpool.tile([1, 128], bf16, name="ones_bf_sb")
                nc.gpsimd.memset(ones_bf_sb[:], 1.0)
            bvrow_sb = bohrow_sb = None
            if with_bv:
                bvrow_sb = cpool.tile([1, 512], bf16, name="bvrow_sb")
                nc.scalar.dma_start(out=bvrow_sb[:],
                                    in_=bias_d["bv"].unsqueeze(0))
            if with_bo:
                bohrow_sb = cpool.tile([1, 512], f32r, name="bohrow_sb")
                nc.scalar.dma_start(out=bohrow_sb[:],
                                    in_=bias_d["bo"].unsqueeze(0))

            def w_qk_slice(name, c, P):
                v = w8[name].rearrange("p (c P t m) -> p c P t m",
                                       c=4, P=2, m=PM)
                return v[:, c, P, :, 0:128]

            def wo_slice(name, Q):
                v = w8[name].rearrange("p (Q t n) -> p Q t n", Q=2, n=PN)
                return v[:, Q, :, 0:512]

            # ---- per-supergroup emitters (2-stage software pipeline) ----
            def load_x(G):
                """Host pre-transposed x (fp8 hi/lo): one DMA per copy."""
                x8t = []
                for nm, d_ap in (("xph", xh_d), ("xpl", xl_d)):
                    t8 = wpool.tile([128, 4 * PN], fp8, name=nm, tag=nm)
                    nc.sync.dma_start(
                        out=t8.rearrange("p (c t) -> p c t",
                                         t=PN)[:, :, 0:512],
                        in_=d_ap.rearrange("(c p) t -> p c t",
                                           p=128)[:, :, G * 512:(G + 1) * 512])
                    x8t.append(t8)
                return x8t[0], x8t[1]

            def x_pair(t8, P):
                v = t8.rearrange("p (P t n) -> p P t n", P=2, n=PN)
                return v[:, P, :, 0:512]

            def proj_qk_chunk(xh_t, xl_t, wname, bname, dst, c):
                """One 512-token feature chunk of q^T or k^T via 6 fp8
                DoubleRow matmuls (hi*hi + hi*lo + lo*hi)."""
                pj_ps = ppool.tile([128, 512], f32, name="pj_ps", tag="ps")
                for P in range(2):
                    prods = ((f"{wname}h", xh_t), (f"{wname}h", xl_t),
                             (f"{wname}l", xh_t))
                    for i, (wn, xt8) in enumerate(prods):
                        nc.tensor.matmul(
                            pj_ps[:],
                            w_qk_slice(wn, c, P),
                            x_pair(xt8, P),
                            start=(P == 0 and i == 0),
                            stop=(P == 1 and i == 2),
                            perf_mode=DR)
                if bname in bias_sb:
                    sc = 1.0 / 1024.0 if wname == "wq" else 1.0
                    nc.scalar.activation(
                        dst[c][:], pj_ps[:], AFT.Identity,
                        bias=bias_sb[bname][:, c:c + 1], scale=sc)
                elif wname == "wq":
                    if c % 2 == 0:
                        nc.vector.tensor_scalar_mul(dst[c][:], pj_ps[:],
                                                    1.0 / 1024.0)
                    else:
                        nc.scalar.activation(dst[c][:], pj_ps[:],
                                             AFT.Identity,
                                             scale=1.0 / 1024.0)
                elif c % 2 == 0:
                    nc.vector.tensor_copy(dst[c][:], pj_ps[:])
                else:
                    nc.scalar.copy(dst[c][:], pj_ps[:])

            def x_tok_slice(t8, P, g):
                v = t8.rearrange("p (P t n) -> p P t n", P=2, n=PN)
                return v[:, P, :, g * 128:(g + 1) * 128]

            def proj_v(xh_t, xl_t):
                v = [wpool.tile([128, 512], bf16, name=f"v{g}", tag=f"v{g}")
                     for g in range(4)]
                for g in range(4):
                    v_ps = ppool.tile([128, 512], f32, name="v_ps", tag="ps")
                    for P in range(2):
                        prods = ((xh_t, "wvh"), (xh_t, "wvl"), (xl_t, "wvh"))
                        for i, (xt8, wn) in enumerate(prods):
                            nc.tensor.matmul(
                                v_ps[:],
                                x_tok_slice(xt8, P, g),
                                wo_slice(wn, P),
                                start=(P == 0 and i == 0),
                                stop=(P == 1 and i == 2 and not with_bv),
                                perf_mode=DR)
                    if with_bv:
                        nc.tensor.matmul(v_ps[:], ones_bf_sb[:],
                                         bvrow_sb[:],
                                         start=False, stop=True)
                    if g % 2 == 0:
                        nc.vector.tensor_scalar_mul(v[g][:], v_ps[:],
                                                    1.0 / 512.0)
                    else:
                        nc.scalar.activation(v[g][:], v_ps[:], AFT.Identity,
                                             scale=1.0 / 512.0)
                return v

            def scores(P, qt, kt, g):
                """S' matmuls + sigmoid + mask for one 128-token group."""
                p4 = []
                for half in range(2):  # even heads / odd heads
                    s_ps = ppool.tile([128, 512], f32, name="s_ps", tag="s",
                                      bufs=4)
                    lo = half * 64
                    for hh in range(4):
                        h = 2 * hh + half
                        c = h // 2
                        gcols = slice(g * 128, (g + 1) * 128)
                        nc.tensor.matmul(
                            s_ps[:, hh * 128:(hh + 1) * 128],
                            kt[c][lo:lo + 64, gcols],
                            qt[c][lo:lo + 64, gcols],
                            start=True, stop=True)
                    p_t = wpool.tile([128, 512], bf16, name=f"p{g}_{half}",
                                     tag=f"p{g}_{half}")
                    nc.scalar.activation(p_t[:], s_ps[:], AFT.Sigmoid)
                    nc.vector.tensor_mul(
                        p_t.rearrange("p (hh t) -> p hh t", hh=4),
                        p_t.rearrange("p (hh t) -> p hh t", hh=4),
                        mask_sb.rearrange("p (hh t) -> p hh t", hh=4))
                    p4.append(p_t)
                return p4

            def ctx_out(P, pr, v):
                ctx8 = []
                for g in range(4):
                    ctx_ps = ppool.tile([128, 512], f32, name="ctx_ps",
                                        tag="ps")
                    for h in range(HEADS):
                        c, lo = h // 2, (h % 2) * 64
                        nc.tensor.matmul(
                            ctx_ps[lo:lo + 64, c * 128:(c + 1) * 128],
                            v[g][:, h * 64:(h + 1) * 64],
                            pr[g][h % 2][:, (h // 2) * 128:
                                          (h // 2 + 1) * 128],
                            start=True, stop=True)
                    # fp8 hi + residual lo in DoubleRow stationary layout
                    c_h = wpool.tile([128, 2 * 2 * PM], fp8, name="cth",
                                     tag=f"cth{g}", bufs=2)
                    c_l = wpool.tile([128, 2 * 2 * PM], fp8, name="ctl",
                                     tag=f"ctl{g}", bufs=2)
                    hv = c_h.rearrange("p (Q t m) -> p Q t m",
                                       Q=2, m=PM)[:, :, :, 0:128]
                    lv = c_l.rearrange("p (Q t m) -> p Q t m",
                                       Q=2, m=PM)[:, :, :, 0:128]
                    src = ctx_ps.rearrange("p (Q t q) -> p Q t q", Q=2, t=2)
                    if g % 2 == 0:
                        nc.scalar.copy(hv, src)
                    else:
                        nc.vector.tensor_copy(hv, src)
                    nc.vector.tensor_tensor(lv, src, hv, SUB)
                    ctx8.append((c_h, c_l))
                for g in range(4):
                    o_ps = ppool.tile([128, 512], f32, name="o_ps", tag="ps")
                    c_h, c_l = ctx8[g]
                    for Q in range(2):
                        prods = ((c_h, "woh"), (c_h, "wol"), (c_l, "woh"))
                        for i, (ct, wn) in enumerate(prods):
                            st = ct.rearrange("p (Q t m) -> p Q t m",
                                              Q=2, m=PM)[:, Q, :, 0:128]
                            nc.tensor.matmul(
                                o_ps[:], st, wo_slice(wn, Q),
                                start=(Q == 0 and i == 0),
                                stop=(Q == 1 and i == 2 and not with_bo),
                                perf_mode=DR)
                    if with_bo:
                        nc.tensor.matmul(o_ps[:], ones_sb[:], bohrow_sb[:],
                                         start=False, stop=True)
                    o_t = wpool.tile([128, 512], bf16, name="o_t", tag="o_t",
                                     bufs=4)
                    if g % 2 == 0:
                        nc.scalar.copy(o_t[:], o_ps[:])
                    else:
                        nc.vector.tensor_copy(o_t[:], o_ps[:])
                    nc.scalar.dma_start(
                        out=y_d[(P * 4 + g) * 128:(P * 4 + g + 1) * 128, :],
                        in_=o_t[:])

            # ---- pipelined emission: stage A(G) interleaved with B(G-1) ----
            x_next = load_x(0)
            prev = None  # (P, qt, kt, v)
            for G in range(NG + 1):
                if G < NG:
                    xh_t, xl_t = x_next
                if G + 1 < NG:
                    x_next = load_x(G + 1)
                pr = []
                if G < NG:
                    qt = [wpool.tile([128, 512], bf16, name=f"wqt{c}",
                                     tag=f"wqt{c}") for c in range(4)]
                    kt = [wpool.tile([128, 512], bf16, name=f"wkt{c}",
                                     tag=f"wkt{c}") for c in range(4)]
                for g in range(4):
                    if prev is not None:
                        pr.append(scores(prev[0], prev[1], prev[2], g))
                    if G < NG:
                        proj_qk_chunk(xh_t, xl_t, "wq", "bq", qt, g)
                        proj_qk_chunk(xh_t, xl_t, "wk", "bk", kt, g)
                if G < NG:
                    v = proj_v(xh_t, xl_t)
                if prev is not None and pr:
                    ctx_out(prev[0], pr, prev[3])
                prev = (G, qt, kt, v) if G < NG else None

    nc.compile()
    return nc


def _get_nc(n_cores, flags):
    key = (n_cores, flags)
    if key not in _CACHE:
        _CACHE[key] = _build(n_cores, *flags)
    return _CACHE[key]


def _mask4():
    m = np.zeros((128, 128), dtype=ml_dtypes.bfloat16)
    for w in range(8):
        m[w * 16:(w + 1) * 16, w * 16:(w + 1) * 16] = 1
    return np.ascontiguousarray(np.tile(m, (1, 4)))


def _fp8_split(a, np8):
    hi = a.astype(np8)
    lo = (a - hi.astype(np.float32)).astype(np8)
    return hi, lo


def _pack_wqk(w, np8):
    """[512, 512] -> [128, (c4, P2, t2, PM)] DoubleRow stationary layout."""
    out = np.zeros((128, 4, 2, 2, PM), np.float32)
    for c in range(4):
        for P in range(2):
            for t in range(2):
                blk = w[(2 * P + t) * 128:(2 * P + t + 1) * 128,
                        c * 128:(c + 1) * 128]
                out[:, c, P, t, 0:128] = blk
    return _fp8_split(np.ascontiguousarray(out.reshape(128, -1)), np8)


def _pack_wo(w):
    """[512, 512] -> [128, (Q2, t2, PN)] DoubleRow moving layout."""
    out = np.zeros((128, 2, 2, PN), np.float32)
    for Q in range(2):
        for t in range(2):
            out[:, Q, t, 0:512] = w[(2 * Q + t) * 128:(2 * Q + t + 1) * 128, :]
    return np.ascontiguousarray(out.reshape(128, -1))


def host_prep(x, Wq, bq, Wk, bk, Wv, bv, Wo, bo, flags):
    """Build the per-core device input dicts."""
    import concourse.mybir as mybir

    np8 = mybir.dt.np(mybir.dt.float8e4)
    # feature-major (pre-transposed) x copies
    xT = np.ascontiguousarray(
        np.asarray(x, np.float32).reshape(N_CORES, TOK, D)
        .transpose(0, 2, 1))                       # [cores, D, TOK]
    xh = xT.astype(np8)
    xl = (xT - xh.astype(np.float32)).astype(np8)

    wqh, wql = _pack_wqk(np.asarray(Wq, np.float32) * (SCALE * WQK_S), np8)
    wkh, wkl = _pack_wqk(np.asarray(Wk, np.float32) * WQK_S, np8)
    woh, wol = _fp8_split(_pack_wo(np.asarray(Wo, np.float32) * WO_S), np8)
    wvh, wvl = _fp8_split(_pack_wo(np.asarray(Wv, np.float32) * WQK_S), np8)
    base = {
        "wqh": wqh, "wql": wql, "wkh": wkh, "wkl": wkl,
        "woh": woh, "wol": wol, "wvh": wvh, "wvl": wvl,
        "maskp": _mask4(),
    }
    for name, b, used, scale in (("bq", bq, flags[0], SCALE),
                                 ("bk", bk, flags[1], WQK_S),
                                 ("bv", bv, flags[2], 512.0 * WV_S),
                                 ("bo", bo, flags[3], 1.0)):
        if used:
            arr = np.asarray(b, np.float32) * scale
            if name == "bv":
                arr = arr.astype(ml_dtypes.bfloat16)
            base[name] = np.ascontiguousarray(arr)

    return [dict(base, xh=np.ascontiguousarray(xh[i]),
                 xl=np.ascontiguousarray(xl[i]))
            for i in range(N_CORES)]


def kernel(x, Wq, bq, Wk, bk, Wv, bv, Wo, bo):
    from concourse.bass_utils import run_bass_kernel_spmd

    in_dt = x.dtype
    flags = tuple(bool(np.any(b)) for b in (bq, bk, bv, bo))
    nc = _get_nc(N_CORES, flags)
    in_maps = host_prep(x, Wq, bq, Wk, bk, Wv, bv, Wo, bo, flags)
    res = run_bass_kernel_spmd(nc, in_maps, list(range(N_CORES)))
    out = np.concatenate([res.results[i]["y"] for i in range(N_CORES)], axis=0)
    return (out.reshape(B, NH, NW, T, D).astype(np.float32)
            .astype(in_dt, copy=False))
